# revision 27
# baseline (speedup 1.0000x reference)
"""Distributed ring-attention kernel for Trainium2 (8 NeuronCores, Bass/Tile).

Strategy (seq-parallel attention, full softmax without max-subtraction):
  - Host: transpose/cast inputs to bf16; shard x.T column-wise (seq) across 8 cores.
  - Per core: project Q/K/V for its 512-seq shard; AllGather K^T and V
    across cores; compute full attention for its Q shard over the whole
    4096-length K/V; out-projection; write its y shard.
  - Collective schedule (the critical path): ncfw boots ~21us in and runs a
    ~25-50us comm-init barrier before the first gather can move, so the
    attention pipeline cannot start before ~85us no matter what. Given that,
    the K/V exchange is split into per-unit AllGathers sized to keep the
    supply ahead of softmax consumption (~27us/pair on the Scalar engine):
    unit 0 (pair 0) ships kT and V as two small ops so scores start ASAP;
    later units ship [kT | V] packed in one flat buffer per op to amortize
    the ~13us fixed ncfw per-op cost.
  - Scores are computed transposed (S^T = K @ Q^T, kpos on partitions) so the
    exp'd probabilities feed the P@V matmul directly as the stationary-side
    contraction. Softmax denominator comes for free from a ones-column
    appended to V. Softmax skips max-subtraction: scores are O(1) here
    (exp is numerically safe), which matches softmax exactly in exact math.
  - Steady state is bound by the Scalar engine: exp of 33.5M scores/core at
    1 elem/lane/cycle @1.2GHz is a ~250us floor (no other engine can read
    PSUM and do transcendentals); PE runs ~70% busy underneath it.
"""

import numpy as np
import ml_dtypes

HID = 1024
HEADS = 16
HD = 64
S = 4096
NCORES = 8
SQ = S // NCORES          # 512 q rows per core
PAIRS = HEADS // 2        # 8 head pairs (128 rows of qkvT per pair)
KTILES = S // 128         # 32 kpos tiles per head
VAUG = HD + 1             # 65: V plus ones column
SCALE = 1.0 / np.sqrt(HD)

_cache = {}


def _build():
    import concourse.bass as bass
    import concourse.mybir as mybir
    import concourse.tile as tile
    from concourse import bacc

    dt = mybir.dt
    nc = bacc.Bacc("TRN2", target_bir_lowering=False, debug=False,
                   num_devices=NCORES)

    xT = nc.dram_tensor("xT", [HID, SQ], dt.bfloat16, kind="ExternalInput").ap()
    wqkvT = nc.dram_tensor("wqkvT", [HID, 3 * HID], dt.bfloat16,
                           kind="ExternalInput").ap()
    woutT = nc.dram_tensor("woutT", [HID, HID], dt.bfloat16,
                           kind="ExternalInput").ap()
    y = nc.dram_tensor("y", [SQ, HID], dt.float32, kind="ExternalOutput").ap()

    with tile.TileContext(nc) as tc:
        _body(nc, tc, bass, mybir, xT, wqkvT, woutT, y)

    nc.compile()
    return nc


def _body(nc, tc, bass, mybir, xT, wqkvT, woutT, y):
    dt = mybir.dt
    f32, bf16, f8 = dt.float32, dt.bfloat16, dt.float8e4
    RG = [list(range(NCORES))]

    with (
        tc.tile_pool(name="dram", bufs=1, space="DRAM") as dram,
        tc.tile_pool(name="resident", bufs=1) as res,
        tc.tile_pool(name="stream", bufs=1) as st,
    ):
        # ---- DRAM bounce buffers for collectives, one set per UNIT of
        # head pairs; first units are single pairs so attention can start
        # as soon as possible ----
        UNITS = [[0], [1], [2, 3], [4, 5], [6, 7]]
        # only unit 0 (the latency-critical head of the attention pipeline)
        # uses split kt/v collectives; unit 1 onward combine kt+v per op to
        # amortize the ~13us fixed ncfw per-op cost
        SPLIT_UNITS = {0}
        unit_of = {}
        for u, prs in enumerate(UNITS):
            for i, p in enumerate(prs):
                unit_of[p] = (u, i)
        # flat combined bounce buffer per unit: [kt strip | v strip], each
        # n*128*SQ elements, so ONE AllGather moves both (bass forbids
        # multi-tensor collectives). Singleton units (the latency-critical
        # first two) instead use SEPARATE kt/v buffers + two collectives so
        # scores can start on kt before v arrives.
        ktvb, ktvg = [], []
        ktb1, ktg1, vb1, vg1 = {}, {}, {}, {}
        for u, prs in enumerate(UNITS):
            n = len(prs)
            if u in (0,):
                ktb1[u] = dram.tile([1, 128 * SQ], bf16, name=f"ktb1_{u}")
                ktg1[u] = dram.tile([NCORES, 128 * SQ], bf16,
                                    addr_space="Shared", name=f"ktg1_{u}")
                vb1[u] = dram.tile([1, 128 * SQ], bf16, name=f"vb1_{u}")
                vg1[u] = dram.tile([NCORES, 128 * SQ], bf16,
                                   addr_space="Shared", name=f"vg1_{u}")
                ktvb.append(None)
                ktvg.append(None)
            else:
                ktvb.append(dram.tile([1, 2 * n * 128 * SQ], bf16,
                                      name=f"ktvb{u}"))
                ktvg.append(dram.tile([NCORES, 2 * n * 128 * SQ], bf16,
                                      addr_space="Shared", name=f"ktvg{u}"))

        # rank register: this core's position in the replica group. Used to
        # ROTATE the gathered-buffer reads so "my own shard" is compile-time
        # block 0 (handled locally, never re-read) and remote shard j comes
        # from gathered block (rank+j)&7 via register-offset SWDGE DMAs.
        rank = nc.gpsimd.cc_rank(replica_groups=RG)
        rot = [None] + [(rank + j) & 7 for j in range(1, NCORES)]

        # ---- load xT (hidden x local-seq), 8 resident tiles ----
        xt = []
        for k in range(8):
            t = res.tile([128, SQ], bf16, tag=f"xt{k}", name=f"xt{k}")
            nc.sync.dma_start(t[:], xT[k * 128:(k + 1) * 128, :])
            xt.append(t)

        # wqkvT strip views for batched weight loads
        wq4 = wqkvT.rearrange("(k p) (m c) -> p m k c", p=128, c=128)
        wv4 = wqkvT.rearrange("(k p) (m c) -> p m k c", p=128, c=512)

        # per-pair resident local K^T strips and per-unit local V+ones
        # blocks ((i, hh, tl, 65) layout) for the early local-shard attention
        ktl = [res.tile([128, SQ], bf16, tag=f"ktl{p}", name=f"ktl{p}")
               for p in range(PAIRS)]
        valu = []
        for u, prs in enumerate(UNITS):
            n = len(prs)
            t = res.tile([128, n * 2 * 4 * VAUG], bf16, tag=f"valu{u}",
                         name=f"valu{u}")
            nc.vector.memset(t[:], 1.0)
            valu.append(t)
        pvloc = [[res.tile([VAUG, SQ], bf16, tag=f"pvloc{p}_{e}",
                           name=f"pvloc{p}_{e}") for e in range(2)]
                 for p in range(PAIRS)]

        def kt_proj(m, psP):
            """K^T rows for pair m (qkvT rows 1024+m*128) -> its unit."""
            u, i = unit_of[m]
            ws = st.tile([128, 8 * 128], bf16, tag="wl", bufs=4)
            nc.sync.dma_start(ws.rearrange("p (k c) -> p k c", c=128),
                              wq4[:, 8 + m, :, :])
            ps = psP.tile([128, SQ], f32, tag="proj", bufs=2)
            for k in range(8):
                nc.tensor.matmul(ps[:], ws[:, k * 128:(k + 1) * 128],
                                 xt[k][:], start=(k == 0), stop=(k == 7))
            nc.vector.tensor_copy(ktl[m][:], ps[:])
            n = len(UNITS[u])
            if u in SPLIT_UNITS:
                nc.sync.dma_start(
                    ktb1[u].rearrange("one (r q) -> one r q", q=SQ)[0],
                    ktl[m][:])
                nc.gpsimd.collective_compute(
                    "AllGather", mybir.AluOpType.bypass, replica_groups=RG,
                    ins=[ktb1[u].opt()], outs=[ktg1[u].opt()])
            else:
                ktpart = ktvb[u].rearrange("one (two i r q) -> one two i r q",
                                           two=2, i=n, r=128, q=SQ)
                nc.sync.dma_start(ktpart[0, 0, i], ktl[m][:])

        wv2 = wqkvT.rearrange("(k p) (m c) -> p m k c", p=128, c=128)

        def v_proj(u, psP):
            """V rows (natural [s,d]) for unit u's pairs -> vb[u] + AG.

            One psum group per s-tile, N = 128 * n_pairs (<=512)."""
            prs = UNITS[u]
            n = len(prs)
            wvs = st.tile([128, 8 * n * 128], bf16, tag="wvs", bufs=3)
            wvs3 = wvs.rearrange("p (k c) -> p k c", c=n * 128)
            nc.sync.dma_start(
                wvs3.rearrange("p k (pr c) -> p k pr c", c=128),
                wv2[:, 16 + prs[0]:16 + prs[0] + n, :, :].rearrange(
                    "p pr k c -> p k pr c"))
            val5 = valu[u].rearrange("q (i hh tl v) -> q i hh tl v",
                                     i=n, hh=2, v=VAUG)
            for sti in range(4):
                ps = psP.tile([128, n * 128], f32, tag="proj", bufs=2)
                for k in range(8):
                    nc.tensor.matmul(
                        ps[:], xt[k][:, sti * 128:(sti + 1) * 128],
                        wvs[:, k * n * 128:(k + 1) * n * 128],
                        start=(k == 0), stop=(k == 7))
                sb = st.tile([128, n * 128], bf16, tag="kv_stage", bufs=4)
                nc.vector.tensor_copy(sb[:], ps[:])
                # local V+ones copy for the early local-shard attention
                nc.vector.tensor_copy(
                    val5[:, :, :, sti, 0:HD],
                    ps.rearrange("q (i hh d) -> q i hh d", hh=2, d=HD))
                if u in SPLIT_UNITS:
                    vpart = vb1[u].rearrange("one (tl q c) -> one tl q c",
                                             tl=4, q=128, c=128)
                    nc.sync.dma_start(vpart[0, sti], sb[:])
                else:
                    vpart = ktvb[u].rearrange(
                        "one (two tl q c) -> one two tl q c",
                        two=2, tl=4, q=128, c=n * 128)
                    nc.sync.dma_start(vpart[0, 1, sti], sb[:])
            if u in SPLIT_UNITS:
                nc.gpsimd.collective_compute(
                    "AllGather", mybir.AluOpType.bypass, replica_groups=RG,
                    ins=[vb1[u].opt()], outs=[vg1[u].opt()])
            else:
                # one combined collective per unit: kT + V strips together
                nc.gpsimd.collective_compute(
                    "AllGather", mybir.AluOpType.bypass, replica_groups=RG,
                    ins=[ktvb[u].opt()], outs=[ktvg[u].opt()])

        qt = [None] * PAIRS

        def q_proj(m, psP):
            ws = st.tile([128, 8 * 128], bf16, tag="wl", bufs=4)
            nc.sync.dma_start(ws.rearrange("p (k c) -> p k c", c=128),
                              wq4[:, m, :, :])
            ps = psP.tile([128, SQ], f32, tag="proj", bufs=2)
            for k in range(8):
                nc.tensor.matmul(ps[:], ws[:, k * 128:(k + 1) * 128],
                                 xt[k][:], start=(k == 0), stop=(k == 7))
            t = res.tile([128, SQ], bf16, tag=f"qt{m}", name=f"qt{m}")
            nc.vector.tensor_copy(t[:], ps[:])
            qt[m] = t

        def local_attn(p, pool, sc_tag="scl", pv_tag="pvl", gsz=2):
            """Attention of this core's Q shard against its OWN K/V shard.
            For pairs 0-3 this runs during the projection phase (psL pool,
            before any gather lands); pairs 4-7 are emitted BETWEEN remote
            pairs as ScalarE filler that absorbs collective-supply gaps,
            reusing the remote pool's sc/pv tags (PSUM is full otherwise).
            The partial PV+l accumulators park in SBUF (pvloc); the remote
            phase adds them back in. Net effect: 1/8 of the exp work moves
            off the ScalarE critical path into its idle windows."""
            u, i = unit_of[p]
            n = len(UNITS[u])
            pvl = [pool.tile([128, SQ], f32, tag=pv_tag, bufs=2,
                             name=f"pvl{p}_{e}") for e in range(2)]
            lslots = [(tl, e) for tl in range(4) for e in range(2)]
            lgroups = [lslots[gs:gs + gsz]
                       for gs in range(0, len(lslots), gsz)]
            lpts = []
            for g in lgroups:
                sc = pool.tile([128, 512 * gsz], f32, tag=sc_tag, bufs=2)
                for idx, (tl, e) in enumerate(g):
                    nc.tensor.matmul(
                        sc[:, idx * 512:(idx + 1) * 512],
                        ktl[p][e * 64:(e + 1) * 64,
                               tl * 128:(tl + 1) * 128],
                        qt[p][e * 64:(e + 1) * 64, :],
                        start=True, stop=True, tile_position=(e * 64, 0))
                pt = st.tile([128, 512 * gsz], bf16, tag="ptl", bufs=5)
                gw = 512 * len(g)
                nc.scalar.activation(pt[:, 0:gw], sc[:, 0:gw],
                                     mybir.ActivationFunctionType.Exp,
                                     scale=float(SCALE))
                lpts.append(pt)
            for gi, g in enumerate(lgroups):
                for idx, (tl, e) in enumerate(g):
                    blk = (i * 2 + e) * 4 + tl
                    nc.tensor.matmul(
                        pvl[e][0:VAUG, :],
                        valu[u][:, blk * VAUG:(blk + 1) * VAUG],
                        lpts[gi][:, idx * 512:(idx + 1) * 512],
                        start=(tl == 0), stop=(tl == 3))
            for e in range(2):
                nc.vector.tensor_copy(pvloc[p][e][:], pvl[e][0:VAUG, :])

        with (
            tc.tile_pool(name="psP", bufs=1, space="PSUM") as psP,
            tc.tile_pool(name="psL", bufs=1, space="PSUM") as psL,
        ):
            # collectives staged/fired in attention consumption order; local
            # attention for each pair interleaves as soon as its projections
            # exist, filling ScalarE's otherwise-dead startup window
            kt_proj(0, psP)
            v_proj(0, psP)
            kt_proj(1, psP)
            v_proj(1, psP)
            q_proj(0, psP)
            local_attn(0, psL)
            q_proj(1, psP)
            local_attn(1, psL)
            kt_proj(2, psP)
            kt_proj(3, psP)
            v_proj(2, psP)
            q_proj(2, psP)
            local_attn(2, psL)
            q_proj(3, psP)
            local_attn(3, psL)
            kt_proj(4, psP)
            kt_proj(5, psP)
            v_proj(3, psP)
            q_proj(4, psP)
            q_proj(5, psP)
            kt_proj(6, psP)
            kt_proj(7, psP)
            v_proj(4, psP)
            q_proj(6, psP)
            q_proj(7, psP)

        # ---- attention (head pairs row-packed on the PE array) ----
        # pair slabs: rows 0..63 even head, 64..127 odd head
        attn = []
        for p in range(PAIRS):
            t = res.tile([128, SQ], bf16, tag=f"attn{p}", name=f"attn{p}")
            attn.append(t)

        # out-projection weights (pair-stacked rows: odd heads at
        # partitions 64..127) and SBUF accumulators for incremental y
        wo5 = woutT.rearrange("(pp r) (o c) -> r o pp c", r=128, c=512)
        wo = []
        for och in range(2):
            w = res.tile([128, PAIRS * 512], bf16, tag=f"wo{och}",
                         name=f"wo{och}")
            nc.sync.dma_start(
                w.rearrange("r (pp c) -> r pp c", c=512), wo5[:, och])
            wo.append(w)
        KR = KTILES - KTILES // NCORES   # 28 remote kpos tiles per pair
        with tc.tile_pool(name="psA", bufs=1, space="PSUM") as psA:
            def remote_attn(p):
                u, i = unit_of[p]
                n = len(UNITS[u])
                # REMOTE pair K^T strip [128, 28*128]: block j-1 holds the
                # shard of peer (rank+j)&7 — rank-rotated register-offset
                # SWDGE DMAs; the local shard (j=0) was handled in the
                # projection phase and is never re-read.
                kth = st.tile([128, KR * 128], bf16, tag="kth", bufs=2)
                if u in SPLIT_UNITS:
                    ktv = ktg1[u].rearrange("c (r q) -> c r q", r=128, q=SQ)
                    vvv = vg1[u].rearrange("c (tl q hh d) -> c hh q tl d",
                                           tl=4, q=128, hh=2, d=HD)
                    kt_of = lambda j: ktv[j]
                    v_of = lambda j, e: vvv[j, e]
                else:
                    ktv = ktvg[u].rearrange("c (two i r q) -> c two i r q",
                                            two=2, i=n, r=128, q=SQ)
                    vvv = ktvg[u].rearrange(
                        "c (two tl q i hh d) -> c two i hh q tl d",
                        two=2, tl=4, q=128, i=n, hh=2, d=HD)
                    kt_of = lambda j: ktv[j, 0, i]
                    v_of = lambda j, e: vvv[j, 1, i, e]
                # row stride in whatever units AP.offset uses, derived
                # empirically so element-vs-byte conventions can't bite
                krow = kt_of(1).offset - kt_of(0).offset
                vrow = v_of(1, 0).offset - v_of(0, 0).offset
                for j in range(1, NCORES):
                    aj = kt_of(j)
                    inner = aj.offset - j * krow
                    sym = bass.AP(aj.tensor, rot[j] * krow + inner, aj.ap,
                                  dep_tracking_offset=aj.offset)
                    nc.gpsimd.dma_start(kth[:, (j - 1) * SQ:j * SQ], sym)
                vah = []
                for e in range(2):
                    # memset 1.0 first: the data DMAs then fill cols 0..63 of
                    # each 65-wide block, leaving col 64 as the ones column.
                    va = st.tile([128, KR * VAUG], bf16, tag="vah",
                                 bufs=6)
                    nc.vector.memset(va[:], 1.0)
                    va4 = va.rearrange("q (c tl v) -> q c tl v",
                                       tl=4, v=VAUG)
                    for j in range(1, NCORES):
                        aj = v_of(j, e)
                        inner = aj.offset - j * vrow
                        sym = bass.AP(aj.tensor, rot[j] * vrow + inner,
                                      aj.ap,
                                      dep_tracking_offset=aj.offset)
                        nc.gpsimd.dma_start(va4[:, j - 1, :, 0:HD], sym)
                    vah.append(va)

                pv = [psA.tile([128, 512], f32, tag="pv", bufs=2,
                               name=f"pv{p}_{e}") for e in range(2)]

                # slot stream: (t, even), (t, odd) pairs; exp groups of 3.
                # Emit the PV matmuls RA groups behind the score/exp stream:
                # the in-order PE queue then has independent scores work
                # ahead of the first PV, which may wait on the V gather.
                RA = 7
                slots = [(t, e) for t in range(KR) for e in range(2)]
                groups = [slots[gs:gs + 3]
                          for gs in range(0, len(slots), 3)]
                pts = []

                def emit_scores(group):
                    gw = 512 * len(group)
                    sc = psA.tile([128, 1536], f32, tag="sc", bufs=2)
                    for idx, (t, e) in enumerate(group):
                        nc.tensor.matmul(
                            sc[:, idx * 512:(idx + 1) * 512],
                            kth[e * 64:(e + 1) * 64, t * 128:(t + 1) * 128],
                            qt[p][e * 64:(e + 1) * 64, :],
                            start=True, stop=True,
                            tile_position=(e * 64, 0))
                    pt = st.tile([128, 1536], bf16, tag="pt", bufs=RA + 2)
                    nc.scalar.activation(pt[:, 0:gw], sc[:, 0:gw],
                                         mybir.ActivationFunctionType.Exp,
                                         scale=float(SCALE))
                    pts.append(pt)

                def emit_pv(group, pt):
                    for idx, (t, e) in enumerate(group):
                        nc.tensor.matmul(
                            pv[e][0:VAUG, :],
                            vah[e][:, t * VAUG:(t + 1) * VAUG],
                            pt[:, idx * 512:(idx + 1) * 512],
                            start=(t == 0), stop=(t == KR - 1))

                for gi, group in enumerate(groups):
                    emit_scores(group)
                    if gi >= RA:
                        emit_pv(groups[gi - RA], pts[gi - RA])
                for gi in range(len(groups) - RA, len(groups)):
                    emit_pv(groups[gi], pts[gi])

                # normalize: out_head = pv_data / l  (l = ones-column row 64).
                # Evacuate pv to SBUF right away (folding in the local-shard
                # partial from the projection phase) so the PSUM slots free
                # for the next pair; the normalize chain runs off-PSUM.
                # (partition_broadcast reads the tile's partition 0, so bounce
                # the l row down to partition 0 via DMA first)
                for e in range(2):
                    pvs = st.tile([VAUG, 512], f32, tag="pvs", bufs=6)
                    nc.vector.tensor_add(pvs[:], pv[e][0:VAUG, :],
                                         pvloc[p][e][:])
                    l0 = st.tile([1, 512], f32, tag="l0", bufs=2)
                    nc.sync.dma_start(l0[:], pvs[64:65, :])
                    lb = st.tile([64, 512], f32, tag="lb", bufs=2)
                    nc.gpsimd.partition_broadcast(lb[:], l0[:])
                    rb = st.tile([64, 512], f32, tag="rb", bufs=2)
                    nc.vector.reciprocal_approx_fast(rb[:], lb[:])
                    if e == 0:
                        nc.vector.tensor_mul(attn[p][0:64, :],
                                             pvs[0:64, :], rb[:])
                    else:
                        ao = st.tile([64, SQ], bf16, tag="ao", bufs=2)
                        nc.vector.tensor_mul(ao[:], pvs[0:64, :], rb[:])
                        nc.gpsimd.dma_start(attn[p][64:128, :], ao[:])

            # ScalarE stream order: remote pairs in gather order, with the
            # late local pairs (4-7, whose projections finish mid-phase)
            # interleaved as filler where the collective supply would
            # otherwise leave ScalarE idle
            remote_attn(0)
            remote_attn(1)
            local_attn(4, psA, sc_tag="sc", pv_tag="pv", gsz=3)
            local_attn(5, psA, sc_tag="sc", pv_tag="pv", gsz=3)
            remote_attn(2)
            local_attn(6, psA, sc_tag="sc", pv_tag="pv", gsz=3)
            remote_attn(3)
            local_attn(7, psA, sc_tag="sc", pv_tag="pv", gsz=3)
            remote_attn(4)
            remote_attn(5)
            remote_attn(6)
            remote_attn(7)

        # ---- out projection: y[s, o] = sum_h attn_h^T.T @ woutT[h rows],
        # row-packed pairs -> two accumulators (even/odd rows), then add ----
        with tc.tile_pool(name="psY", bufs=1, space="PSUM") as psY:
            for sti in range(4):
                for och in range(2):
                    psa = psY.tile([128, 512], f32, tag="ya", bufs=4)
                    for p in range(PAIRS):
                        # full 128-row contraction: both heads of the pair
                        # contribute additively (attn rows match wo rows)
                        nc.tensor.matmul(
                            psa[:], attn[p][:, sti * 128:(sti + 1) * 128],
                            wo[och][:, p * 512:(p + 1) * 512],
                            start=(p == 0), stop=(p == PAIRS - 1))
                    ysb = st.tile([128, 512], f32, tag="ysb", bufs=4)
                    nc.vector.tensor_copy(ysb[:], psa[:])
                    nc.sync.dma_start(
                        y[sti * 128:(sti + 1) * 128,
                          och * 512:(och + 1) * 512], ysb[:])


def _get_nc():
    if "nc" not in _cache:
        _cache["nc"] = _build()
    return _cache["nc"]


def kernel(x, W_qkv, W_out, _trace=False):
    from concourse.bass_utils import run_bass_kernel_spmd

    nc = _get_nc()
    bf16 = ml_dtypes.bfloat16

    x = np.asarray(x)
    xTf = np.ascontiguousarray(x.reshape(S, HID).T).astype(bf16)   # [HID, S]
    wqkvT = np.ascontiguousarray(np.asarray(W_qkv).T).astype(bf16)
    woutT = np.ascontiguousarray(np.asarray(W_out).T).astype(bf16)

    in_maps = []
    for c in range(NCORES):
        in_maps.append({
            "xT": np.ascontiguousarray(xTf[:, c * SQ:(c + 1) * SQ]),
            "wqkvT": wqkvT,
            "woutT": woutT,
        })
    res = run_bass_kernel_spmd(nc, in_maps, core_ids=list(range(NCORES)),
                               trace=_trace)
    out = np.concatenate([res.results[c]["y"] for c in range(NCORES)],
                         axis=0)
    out = out.reshape(1, S, HID).astype(np.float32)
    if _trace:
        kernel.last_results = res
    return out



# revision 31
# speedup vs baseline: 1.0208x; 1.0208x over previous
"""Distributed ring-attention kernel for Trainium2 (8 NeuronCores, Bass/Tile).

Strategy (seq-parallel attention, full softmax without max-subtraction):
  - Host: transpose/cast inputs to bf16; shard x.T column-wise (seq) across 8 cores.
  - Per core: project Q/K/V for its 512-seq shard; AllGather K^T and V
    across cores; compute full attention for its Q shard over the whole
    4096-length K/V; out-projection; write its y shard.
  - Collective schedule (the critical path): ncfw boots ~21us in and runs a
    ~25-50us comm-init barrier before the first gather can move, so the
    attention pipeline cannot start before ~85us no matter what. Given that,
    the K/V exchange is split into per-unit AllGathers sized to keep the
    supply ahead of softmax consumption (~27us/pair on the Scalar engine):
    unit 0 (pair 0) ships kT and V as two small ops so scores start ASAP;
    later units ship [kT | V] packed in one flat buffer per op to amortize
    the ~13us fixed ncfw per-op cost.
  - Scores are computed transposed (S^T = K @ Q^T, kpos on partitions) so the
    exp'd probabilities feed the P@V matmul directly as the stationary-side
    contraction. Softmax denominator comes for free from a ones-column
    appended to V. Softmax skips max-subtraction: scores are O(1) here
    (exp is numerically safe), which matches softmax exactly in exact math.
  - Steady state is bound by the Scalar engine: exp of 33.5M scores/core at
    1 elem/lane/cycle @1.2GHz is a ~250us floor (no other engine can read
    PSUM and do transcendentals); PE runs ~70% busy underneath it.
"""

import numpy as np
import ml_dtypes

HID = 1024
HEADS = 16
HD = 64
S = 4096
NCORES = 8
SQ = S // NCORES          # 512 q rows per core
PAIRS = HEADS // 2        # 8 head pairs (128 rows of qkvT per pair)
KTILES = S // 128         # 32 kpos tiles per head
VAUG = HD + 1             # 65: V plus ones column
SCALE = 1.0 / np.sqrt(HD)

_cache = {}


def _build():
    import concourse.bass as bass
    import concourse.mybir as mybir
    import concourse.tile as tile
    from concourse import bacc

    dt = mybir.dt
    nc = bacc.Bacc("TRN2", target_bir_lowering=False, debug=False,
                   num_devices=NCORES)

    xT = nc.dram_tensor("xT", [HID, SQ], dt.bfloat16, kind="ExternalInput").ap()
    wqkvT = nc.dram_tensor("wqkvT", [HID, 3 * HID], dt.bfloat16,
                           kind="ExternalInput").ap()
    woutT = nc.dram_tensor("woutT", [HID, HID], dt.bfloat16,
                           kind="ExternalInput").ap()
    y = nc.dram_tensor("y", [SQ, HID], dt.float32, kind="ExternalOutput").ap()

    with tile.TileContext(nc) as tc:
        _body(nc, tc, bass, mybir, xT, wqkvT, woutT, y)

    nc.compile()
    return nc


def _body(nc, tc, bass, mybir, xT, wqkvT, woutT, y):
    dt = mybir.dt
    f32, bf16, f8 = dt.float32, dt.bfloat16, dt.float8e4
    RG = [list(range(NCORES))]

    with (
        tc.tile_pool(name="dram", bufs=1, space="DRAM") as dram,
        tc.tile_pool(name="resident", bufs=1) as res,
        tc.tile_pool(name="stream", bufs=1) as st,
    ):
        # ---- DRAM bounce buffers for collectives, one set per UNIT of
        # head pairs; first units are single pairs so attention can start
        # as soon as possible ----
        UNITS = [[0], [1], [2, 3], [4, 5], [6, 7]]
        # only unit 0 (the latency-critical head of the attention pipeline)
        # uses split kt/v collectives; unit 1 onward combine kt+v per op to
        # amortize the ~13us fixed ncfw per-op cost
        SPLIT_UNITS = {0}
        unit_of = {}
        for u, prs in enumerate(UNITS):
            for i, p in enumerate(prs):
                unit_of[p] = (u, i)
        # flat combined bounce buffer per unit: [kt strip | v strip], each
        # n*128*SQ elements, so ONE AllGather moves both (bass forbids
        # multi-tensor collectives). Singleton units (the latency-critical
        # first two) instead use SEPARATE kt/v buffers + two collectives so
        # scores can start on kt before v arrives.
        ktvb, ktvg = [], []
        ktb1, ktg1, vb1, vg1 = {}, {}, {}, {}
        for u, prs in enumerate(UNITS):
            n = len(prs)
            if u in (0,):
                ktb1[u] = dram.tile([1, 128 * SQ], bf16, name=f"ktb1_{u}")
                ktg1[u] = dram.tile([NCORES, 128 * SQ], bf16,
                                    addr_space="Shared", name=f"ktg1_{u}")
                vb1[u] = dram.tile([1, 128 * SQ], bf16, name=f"vb1_{u}")
                vg1[u] = dram.tile([NCORES, 128 * SQ], bf16,
                                   addr_space="Shared", name=f"vg1_{u}")
                ktvb.append(None)
                ktvg.append(None)
            else:
                ktvb.append(dram.tile([1, 2 * n * 128 * SQ], bf16,
                                      name=f"ktvb{u}"))
                ktvg.append(dram.tile([NCORES, 2 * n * 128 * SQ], bf16,
                                      addr_space="Shared", name=f"ktvg{u}"))

        # rank register: this core's position in the replica group. Used to
        # ROTATE the gathered-buffer reads for pairs 0-3 so their own shard
        # is handled locally during the projection phase and remote shard j
        # comes from gathered block (rank+j)&7 via register-offset SWDGE
        # DMAs. Pairs 4-7 keep the plain full-gather path (their projections
        # finish too late for early-local work to pay off).
        LOCAL_PAIRS = 4
        rank = nc.gpsimd.cc_rank(replica_groups=RG)
        rot = [None] + [(rank + j) & 7 for j in range(1, NCORES)]

        # ---- load xT (hidden x local-seq), 8 resident tiles ----
        xt = []
        for k in range(8):
            t = res.tile([128, SQ], bf16, tag=f"xt{k}", name=f"xt{k}")
            nc.sync.dma_start(t[:], xT[k * 128:(k + 1) * 128, :])
            xt.append(t)

        # wqkvT strip views for batched weight loads
        wq4 = wqkvT.rearrange("(k p) (m c) -> p m k c", p=128, c=128)
        wv4 = wqkvT.rearrange("(k p) (m c) -> p m k c", p=128, c=512)

        # per-pair resident local K^T strips and per-unit local V+ones
        # blocks ((i, hh, tl, 65) layout) for the early local-shard
        # attention of pairs 0..LOCAL_PAIRS-1 (units 0..2)
        LOCAL_UNITS = 3
        ktl = [res.tile([128, SQ], bf16, tag=f"ktl{p}", name=f"ktl{p}")
               for p in range(LOCAL_PAIRS)]
        valu = []
        for u in range(LOCAL_UNITS):
            n = len(UNITS[u])
            t = res.tile([128, n * 2 * 4 * VAUG], bf16, tag=f"valu{u}",
                         name=f"valu{u}")
            nc.vector.memset(t[:], 1.0)
            valu.append(t)
        pvloc = [[res.tile([VAUG, SQ], bf16, tag=f"pvloc{p}_{e}",
                           name=f"pvloc{p}_{e}") for e in range(2)]
                 for p in range(LOCAL_PAIRS)]

        def kt_proj(m, psP):
            """K^T rows for pair m (qkvT rows 1024+m*128) -> its unit."""
            u, i = unit_of[m]
            ws = st.tile([128, 8 * 128], bf16, tag="wl", bufs=4)
            nc.sync.dma_start(ws.rearrange("p (k c) -> p k c", c=128),
                              wq4[:, 8 + m, :, :])
            ps = psP.tile([128, SQ], f32, tag="proj", bufs=2)
            for k in range(8):
                nc.tensor.matmul(ps[:], ws[:, k * 128:(k + 1) * 128],
                                 xt[k][:], start=(k == 0), stop=(k == 7))
            if m < LOCAL_PAIRS:
                sb = ktl[m]
            else:
                sb = st.tile([128, SQ], bf16, tag="kt_stage", bufs=4)
            nc.vector.tensor_copy(sb[:], ps[:])
            n = len(UNITS[u])
            if u in SPLIT_UNITS:
                nc.sync.dma_start(
                    ktb1[u].rearrange("one (r q) -> one r q", q=SQ)[0],
                    sb[:])
                nc.gpsimd.collective_compute(
                    "AllGather", mybir.AluOpType.bypass, replica_groups=RG,
                    ins=[ktb1[u].opt()], outs=[ktg1[u].opt()])
            else:
                ktpart = ktvb[u].rearrange("one (two i r q) -> one two i r q",
                                           two=2, i=n, r=128, q=SQ)
                nc.sync.dma_start(ktpart[0, 0, i], sb[:])

        wv2 = wqkvT.rearrange("(k p) (m c) -> p m k c", p=128, c=128)

        def v_proj(u, psP):
            """V rows (natural [s,d]) for unit u's pairs -> vb[u] + AG.

            One psum group per s-tile, N = 128 * n_pairs (<=512)."""
            prs = UNITS[u]
            n = len(prs)
            wvs = st.tile([128, 8 * n * 128], bf16, tag="wvs", bufs=3)
            wvs3 = wvs.rearrange("p (k c) -> p k c", c=n * 128)
            nc.sync.dma_start(
                wvs3.rearrange("p k (pr c) -> p k pr c", c=128),
                wv2[:, 16 + prs[0]:16 + prs[0] + n, :, :].rearrange(
                    "p pr k c -> p k pr c"))
            for sti in range(4):
                ps = psP.tile([128, n * 128], f32, tag="proj", bufs=2)
                for k in range(8):
                    nc.tensor.matmul(
                        ps[:], xt[k][:, sti * 128:(sti + 1) * 128],
                        wvs[:, k * n * 128:(k + 1) * n * 128],
                        start=(k == 0), stop=(k == 7))
                sb = st.tile([128, n * 128], bf16, tag="kv_stage", bufs=4)
                nc.vector.tensor_copy(sb[:], ps[:])
                if u < LOCAL_UNITS:
                    val5 = valu[u].rearrange(
                        "q (i hh tl v) -> q i hh tl v", i=n, hh=2, v=VAUG)
                    nc.vector.tensor_copy(
                        val5[:, :, :, sti, 0:HD],
                        ps.rearrange("q (i hh d) -> q i hh d", hh=2, d=HD))
                if u in SPLIT_UNITS:
                    vpart = vb1[u].rearrange("one (tl q c) -> one tl q c",
                                             tl=4, q=128, c=128)
                    nc.sync.dma_start(vpart[0, sti], sb[:])
                else:
                    vpart = ktvb[u].rearrange(
                        "one (two tl q c) -> one two tl q c",
                        two=2, tl=4, q=128, c=n * 128)
                    nc.sync.dma_start(vpart[0, 1, sti], sb[:])
            if u in SPLIT_UNITS:
                nc.gpsimd.collective_compute(
                    "AllGather", mybir.AluOpType.bypass, replica_groups=RG,
                    ins=[vb1[u].opt()], outs=[vg1[u].opt()])
            else:
                # one combined collective per unit: kT + V strips together
                nc.gpsimd.collective_compute(
                    "AllGather", mybir.AluOpType.bypass, replica_groups=RG,
                    ins=[ktvb[u].opt()], outs=[ktvg[u].opt()])

        qt = [None] * PAIRS

        def q_proj(m, psP):
            ws = st.tile([128, 8 * 128], bf16, tag="wl", bufs=4)
            nc.sync.dma_start(ws.rearrange("p (k c) -> p k c", c=128),
                              wq4[:, m, :, :])
            ps = psP.tile([128, SQ], f32, tag="proj", bufs=2)
            for k in range(8):
                nc.tensor.matmul(ps[:], ws[:, k * 128:(k + 1) * 128],
                                 xt[k][:], start=(k == 0), stop=(k == 7))
            t = res.tile([128, SQ], bf16, tag=f"qt{m}", name=f"qt{m}")
            nc.vector.tensor_copy(t[:], ps[:])
            qt[m] = t

        def local_attn(p, psL):
            """Attention of this core's Q shard against its OWN K/V shard,
            run during the projection phase before any gather lands. The
            partial PV+l accumulators park in SBUF (pvloc); the remote pass
            for this pair covers only the 7 remote shards (rank-rotated)
            and adds the partial back. Moves 1/8 of pair-p exp work into
            ScalarE's otherwise-dead startup window."""
            u, i = unit_of[p]
            pvl = [psL.tile([128, SQ], f32, tag="pvl", bufs=2,
                            name=f"pvl{p}_{e}") for e in range(2)]
            lgroups = [[(tl, 0), (tl, 1)] for tl in range(4)]
            lpts = []
            for g in lgroups:
                sc = psL.tile([128, 1024], f32, tag="scl", bufs=2)
                for idx, (tl, e) in enumerate(g):
                    nc.tensor.matmul(
                        sc[:, idx * 512:(idx + 1) * 512],
                        ktl[p][e * 64:(e + 1) * 64,
                               tl * 128:(tl + 1) * 128],
                        qt[p][e * 64:(e + 1) * 64, :],
                        start=True, stop=True, tile_position=(e * 64, 0))
                pt = st.tile([128, 1024], bf16, tag="ptl", bufs=5)
                nc.scalar.activation(pt[:], sc[:],
                                     mybir.ActivationFunctionType.Exp,
                                     scale=float(SCALE))
                lpts.append(pt)
            for gi, g in enumerate(lgroups):
                for idx, (tl, e) in enumerate(g):
                    blk = (i * 2 + e) * 4 + tl
                    nc.tensor.matmul(
                        pvl[e][0:VAUG, :],
                        valu[u][:, blk * VAUG:(blk + 1) * VAUG],
                        lpts[gi][:, idx * 512:(idx + 1) * 512],
                        start=(tl == 0), stop=(tl == 3))
            for e in range(2):
                nc.vector.tensor_copy(pvloc[p][e][:], pvl[e][0:VAUG, :])

        with (
            tc.tile_pool(name="psP", bufs=1, space="PSUM") as psP,
            tc.tile_pool(name="psL", bufs=1, space="PSUM") as psL,
        ):
            # emit each unit's kT then V so the collectives fire in exactly
            # the order attention consumes them; local-shard attention for
            # pairs 0-3 interleaves as soon as its projections exist,
            # filling ScalarE's otherwise-dead startup window
            kt_proj(0, psP)
            v_proj(0, psP)
            kt_proj(1, psP)
            v_proj(1, psP)
            q_proj(0, psP)
            local_attn(0, psL)
            q_proj(1, psP)
            local_attn(1, psL)
            kt_proj(2, psP)
            kt_proj(3, psP)
            v_proj(2, psP)
            q_proj(2, psP)
            local_attn(2, psL)
            q_proj(3, psP)
            local_attn(3, psL)
            kt_proj(4, psP)
            kt_proj(5, psP)
            v_proj(3, psP)
            kt_proj(6, psP)
            kt_proj(7, psP)
            v_proj(4, psP)
            for m in range(4, PAIRS):
                q_proj(m, psP)

        # ---- attention (head pairs row-packed on the PE array) ----
        # pair slabs: rows 0..63 even head, 64..127 odd head
        attn = []
        for p in range(PAIRS):
            t = res.tile([128, SQ], bf16, tag=f"attn{p}", name=f"attn{p}")
            attn.append(t)

        # out-projection weights (pair-stacked rows: odd heads at
        # partitions 64..127) and SBUF accumulators for incremental y
        wo5 = woutT.rearrange("(pp r) (o c) -> r o pp c", r=128, c=512)
        wo = []
        for och in range(2):
            w = res.tile([128, PAIRS * 512], bf16, tag=f"wo{och}",
                         name=f"wo{och}")
            nc.sync.dma_start(
                w.rearrange("r (pp c) -> r pp c", c=512), wo5[:, och])
            wo.append(w)
        KR = KTILES - KTILES // NCORES   # 28 remote kpos tiles (local pairs)
        with tc.tile_pool(name="psA", bufs=1, space="PSUM") as psA:
            for p in range(PAIRS):
                u, i = unit_of[p]
                n = len(UNITS[u])
                rotated = p < LOCAL_PAIRS
                NT = KR if rotated else KTILES
                # pair K^T strip: rows 0..63 head 2p, 64..127 head 2p+1 —
                # matches qt[p] halves and tile_position rows. For rotated
                # pairs, block j-1 holds the shard of peer (rank+j)&7 via
                # register-offset SWDGE DMAs (local shard never re-read);
                # full pairs load all 8 blocks in gather order.
                kth = st.tile([128, KTILES * 128], bf16, tag="kth", bufs=2)
                if u in SPLIT_UNITS:
                    ktv = ktg1[u].rearrange("c (r q) -> c r q", r=128, q=SQ)
                    vvv = vg1[u].rearrange("c (tl q hh d) -> c hh q tl d",
                                           tl=4, q=128, hh=2, d=HD)
                    kt_of = lambda j: ktv[j]
                    v_of = lambda j, e: vvv[j, e]
                else:
                    ktv = ktvg[u].rearrange("c (two i r q) -> c two i r q",
                                            two=2, i=n, r=128, q=SQ)
                    vvv = ktvg[u].rearrange(
                        "c (two tl q i hh d) -> c two i hh q tl d",
                        two=2, tl=4, q=128, i=n, hh=2, d=HD)
                    kt_of = lambda j: ktv[j, 0, i]
                    v_of = lambda j, e: vvv[j, 1, i, e]
                krow = kt_of(1).offset - kt_of(0).offset
                vrow = v_of(1, 0).offset - v_of(0, 0).offset
                if rotated:
                    for j in range(1, NCORES):
                        aj = kt_of(j)
                        inner = aj.offset - j * krow
                        sym = bass.AP(aj.tensor, rot[j] * krow + inner,
                                      aj.ap, dep_tracking_offset=aj.offset)
                        nc.gpsimd.dma_start(kth[:, (j - 1) * SQ:j * SQ], sym)
                else:
                    ktg3 = (kt_of(0).tensor.ap() if False else None)
                    kthv = kth.rearrange("r (c q) -> r c q", q=SQ)
                    for j in range(NCORES):
                        nc.sync.dma_start(kthv[:, j, :], kt_of(j))
                vah = []
                for e in range(2):
                    # memset 1.0 first: the data DMAs then fill cols 0..63 of
                    # each 65-wide block, leaving col 64 as the ones column.
                    va = st.tile([128, KTILES * VAUG], bf16, tag="vah",
                                 bufs=6)
                    nc.vector.memset(va[:], 1.0)
                    va4 = va.rearrange("q (c tl v) -> q c tl v",
                                       tl=4, v=VAUG)
                    if rotated:
                        for j in range(1, NCORES):
                            aj = v_of(j, e)
                            inner = aj.offset - j * vrow
                            sym = bass.AP(aj.tensor, rot[j] * vrow + inner,
                                          aj.ap,
                                          dep_tracking_offset=aj.offset)
                            nc.gpsimd.dma_start(va4[:, j - 1, :, 0:HD], sym)
                    else:
                        for j in range(NCORES):
                            nc.gpsimd.dma_start(va4[:, j, :, 0:HD],
                                                v_of(j, e))
                    vah.append(va)

                pv = [psA.tile([128, 512], f32, tag="pv", bufs=2,
                               name=f"pv{p}_{e}") for e in range(2)]

                # slot stream: (t, even), (t, odd) pairs; exp groups of 3.
                # Emit the PV matmuls RA groups behind the score/exp stream:
                # the in-order PE queue then has independent scores work
                # ahead of the first PV, which may wait on the V gather.
                RA = 7
                slots = [(t, e) for t in range(NT) for e in range(2)]
                groups = [slots[gs:gs + 3]
                          for gs in range(0, len(slots), 3)]
                pts = []

                def emit_scores(group):
                    gw = 512 * len(group)
                    sc = psA.tile([128, 1536], f32, tag="sc", bufs=2)
                    for idx, (t, e) in enumerate(group):
                        nc.tensor.matmul(
                            sc[:, idx * 512:(idx + 1) * 512],
                            kth[e * 64:(e + 1) * 64, t * 128:(t + 1) * 128],
                            qt[p][e * 64:(e + 1) * 64, :],
                            start=True, stop=True,
                            tile_position=(e * 64, 0))
                    pt = st.tile([128, 1536], bf16, tag="pt", bufs=RA + 2)
                    nc.scalar.activation(pt[:, 0:gw], sc[:, 0:gw],
                                         mybir.ActivationFunctionType.Exp,
                                         scale=float(SCALE))
                    pts.append(pt)

                def emit_pv(group, pt):
                    for idx, (t, e) in enumerate(group):
                        nc.tensor.matmul(
                            pv[e][0:VAUG, :],
                            vah[e][:, t * VAUG:(t + 1) * VAUG],
                            pt[:, idx * 512:(idx + 1) * 512],
                            start=(t == 0), stop=(t == NT - 1))

                for gi, group in enumerate(groups):
                    emit_scores(group)
                    if gi >= RA:
                        emit_pv(groups[gi - RA], pts[gi - RA])
                for gi in range(len(groups) - RA, len(groups)):
                    emit_pv(groups[gi], pts[gi])

                # normalize: out_head = pv_data / l  (l = ones-column row 64).
                # Evacuate pv to SBUF right away (rotated pairs fold in the
                # local-shard partial) so the PSUM slots free for the next
                # pair; the normalize chain then runs off-PSUM.
                # (partition_broadcast reads the tile's partition 0, so bounce
                # the l row down to partition 0 via DMA first)
                for e in range(2):
                    pvs = st.tile([VAUG, 512], f32, tag="pvs", bufs=6)
                    if rotated:
                        nc.vector.tensor_add(pvs[:], pv[e][0:VAUG, :],
                                             pvloc[p][e][:])
                    else:
                        nc.vector.tensor_copy(pvs[:], pv[e][0:VAUG, :])
                    l0 = st.tile([1, 512], f32, tag="l0", bufs=2)
                    nc.sync.dma_start(l0[:], pvs[64:65, :])
                    lb = st.tile([64, 512], f32, tag="lb", bufs=2)
                    nc.gpsimd.partition_broadcast(lb[:], l0[:])
                    rb = st.tile([64, 512], f32, tag="rb", bufs=2)
                    nc.vector.reciprocal_approx_fast(rb[:], lb[:])
                    if e == 0:
                        nc.vector.tensor_mul(attn[p][0:64, :],
                                             pvs[0:64, :], rb[:])
                    else:
                        ao = st.tile([64, SQ], bf16, tag="ao", bufs=2)
                        nc.vector.tensor_mul(ao[:], pvs[0:64, :], rb[:])
                        nc.gpsimd.dma_start(attn[p][64:128, :], ao[:])

        # ---- out projection: y[s, o] = sum_h attn_h^T.T @ woutT[h rows],
        # row-packed pairs -> two accumulators (even/odd rows), then add ----
        with tc.tile_pool(name="psY", bufs=1, space="PSUM") as psY:
            for sti in range(4):
                for och in range(2):
                    psa = psY.tile([128, 512], f32, tag="ya", bufs=4)
                    for p in range(PAIRS):
                        # full 128-row contraction: both heads of the pair
                        # contribute additively (attn rows match wo rows)
                        nc.tensor.matmul(
                            psa[:], attn[p][:, sti * 128:(sti + 1) * 128],
                            wo[och][:, p * 512:(p + 1) * 512],
                            start=(p == 0), stop=(p == PAIRS - 1))
                    ysb = st.tile([128, 512], f32, tag="ysb", bufs=4)
                    nc.vector.tensor_copy(ysb[:], psa[:])
                    nc.sync.dma_start(
                        y[sti * 128:(sti + 1) * 128,
                          och * 512:(och + 1) * 512], ysb[:])


def _get_nc():
    if "nc" not in _cache:
        _cache["nc"] = _build()
    return _cache["nc"]


def kernel(x, W_qkv, W_out, _trace=False):
    from concourse.bass_utils import run_bass_kernel_spmd

    nc = _get_nc()
    bf16 = ml_dtypes.bfloat16

    x = np.asarray(x)
    xTf = np.ascontiguousarray(x.reshape(S, HID).T).astype(bf16)   # [HID, S]
    wqkvT = np.ascontiguousarray(np.asarray(W_qkv).T).astype(bf16)
    woutT = np.ascontiguousarray(np.asarray(W_out).T).astype(bf16)

    in_maps = []
    for c in range(NCORES):
        in_maps.append({
            "xT": np.ascontiguousarray(xTf[:, c * SQ:(c + 1) * SQ]),
            "wqkvT": wqkvT,
            "woutT": woutT,
        })
    res = run_bass_kernel_spmd(nc, in_maps, core_ids=list(range(NCORES)),
                               trace=_trace)
    out = np.concatenate([res.results[c]["y"] for c in range(NCORES)],
                         axis=0)
    out = out.reshape(1, S, HID).astype(np.float32)
    if _trace:
        kernel.last_results = res
    return out

